# revision 15
# baseline (speedup 1.0000x reference)
"""Causal multi-head attention (B=2, H=16, S=2048, D=128, fp32) on 8 NeuronCores.

Sharding: the 32 (batch, head) pairs are split 4-per-core (tensor parallel over
heads, data parallel over batch — both collapse to the fused pair axis).

Per-core kernel (per pair), flash-attention style without max-subtraction
(scores have unit variance, so exp never overflows in fp32):

  scores_T[k, q] = K_blk^T.T @ Q^T            (bf16 matmuls into fp32 PSUM,
                                               causally trimmed free dim;
                                               Q^T prescaled by SCALE/16 so
                                               PSUM holds u = s/16)
  P_T = exp(s)                                SPLIT between ScalarE (Exp
                                              activation, scale=16) and DVE
                                              (custom EXP16_POLY_ANT op:
                                              deg-2 poly in u then 4
                                              squarings = exp(16u)·(1+2%)),
                                              strips routed by greedy balance;
                                              DVE only gets strips whose
                                              min row has >=512 keys so the
                                              softmax normalization washes
                                              out the poly ripple.
  causal mask on diagonal 128x128 blocks      (a -8 strict-upper-triangular
                                               bias is matmul-accumulated into
                                               the diagonal score block — tri
                                               stationary x identity moving —
                                               so ScalarE exp(16u-128) gives
                                               exact zeros; no mask multiply)
  ctx[q, 0:128] , l[q] = P_T_blk.T @ [V | 1]  (bf16 matmuls, PSUM-accumulated
                                               over k blocks; the ones column
                                               of V_aug yields the softmax
                                               denominator for free)
  out[q, :] = ctx[q, :] / l[q]                (DVE batched reciprocal per PSUM
                                               bank + per-group scalar multiply
                                               assigned to ScalarE or DVE by
                                               running balance)

Scheduling notes: scores for block kb+1 are emitted before PV matmuls of block
kb so the PE FIFO keeps the exp engines fed; alternating strip routing keeps
ScalarE and DVE concurrently busy. PSUM start=True clears has_written for a
whole bank, so of the 8 packed ctx accumulation groups only the first per bank
(s=0/3/6) uses start=True and the rest rely on overwrite-on-first-touch.

Q^T / K^T (bf16) and the bf16 [V | 1] augmentation are prepared host-side in
kernel() — host preprocessing is part of the sharding step.
"""

import math

import ml_dtypes
import numpy as np

import concourse.bass as bass
import concourse.mybir as mybir
from concourse import bacc, tile
from concourse.bass_utils import run_bass_kernel_spmd

# ---- custom DVE op: EXP16_POLY_ANT -----------------------------------------
# out = ((C0*u + C1)*u + C2)^16 ~= exp(16u) for u = s/16, accurate to ~2% for
# |s| <= 4.5 with graceful degradation in the tails (poly is positive
# everywhere: discriminant < 0). Registered into concourse.dve_ops at import.
from concourse.dve_spec import Spec, Src0, C0, C1, C2, lower, sq
from concourse.dve_uop import DveOpSpec
import concourse.dve_ops as dve_ops_mod
from concourse.dve_ops import DveOp

_u = Src0
_body = sq(sq(sq(sq((_u * C0 + C1) * _u + C2))))


def _exp16_ref(in0, in1, s0, s1, imm2):
    u = in0.astype(np.float32)
    p = ((u * np.float32(s0) + np.float32(s1)) * u + np.float32(imm2)).astype(
        np.float32
    )
    for _ in range(4):
        p = (p * p).astype(np.float32)
    return p


_spec = Spec(body=_body, reference=_exp16_ref)
_EXP16_NAME = "EXP16_POLY_ANT"
_EXP16_ROW = 17  # rows [1, 0x20) free; production ops use 1..16
_sha = DveOpSpec(
    name=_EXP16_NAME, opcode=_EXP16_ROW, uops=lower(_spec, ver="v3"), rd1_en=False
).sha("v3")
EXP16 = DveOp(_EXP16_NAME, _spec, subdim=False, uops_sha={"v3": _sha})
if _EXP16_NAME not in dve_ops_mod._SUB_OPCODE_FOR_NAME:
    dve_ops_mod.OPS.append(EXP16)
    dve_ops_mod.CUSTOM_DVE_SPECS[_EXP16_NAME] = _spec
    dve_ops_mod._SUB_OPCODE_FOR_NAME[_EXP16_NAME] = _EXP16_ROW

# minimax-relative fit of exp(u) on |u| <= 4.5/16 (see transcript)
B2, B1, B0 = 0.49894845, 1.0117635, 1.000231

B, H, S, D = 2, 16, 2048, 128
NCORES = 8
NPAIRS = B * H              # 32 fused (batch, head) pairs
PPC = NPAIRS // NCORES      # 4 pairs per core
KB = 128                    # k block (PE contraction / partition dim)
QC = 1024                   # q chunk (scores psum free dim)
NSUB = QC // 128            # sub-q blocks (PV stationary width) per chunk
NKT = S // KB               # 16 k blocks per sequence
SCALE = 1.0 / math.sqrt(D)  # net score scale: /(sqrt(d)*coeff) then *coeff
PRE = SCALE / 16.0          # host-side Q prescale; PSUM holds u = s/16

F32 = mybir.dt.float32
BF16 = mybir.dt.bfloat16

# modeled engine-op costs (ns) for the static balancer
_NS_COL_S = 0.833   # ScalarE activation per free column
_NS_COL_D = 1.042   # DVE per free column
_OV_S = 185.0       # ScalarE per-instruction overhead
_OV_D = 130.0       # DVE per-instruction overhead
_NORM_S = 380.0     # ScalarE Copy+scale [128,128] (measured)
_NORM_D = 322.0     # DVE tensor_scalar_mul [128,128] (measured)


def _build_nc():
    nc = bacc.Bacc("TRN2", target_bir_lowering=False, debug=False)
    qt_d = nc.dram_tensor("qt", [PPC, D, S], BF16, kind="ExternalInput")
    kt_d = nc.dram_tensor("kt", [PPC, D, S], BF16, kind="ExternalInput")
    va_d = nc.dram_tensor("va", [PPC, KB, NKT, KB + 1], BF16, kind="ExternalInput")
    out_d = nc.dram_tensor("out", [PPC, S, D], F32, kind="ExternalOutput")

    # Raw-bass warmup activation before the Tile body: places the ~1.3us ACT
    # table load in the preamble, off the first chunk's critical path.
    warm_sb = nc.alloc_sbuf_tensor("warm_sb", [128, 1], F32)
    nc.scalar.activation(
        warm_sb.ap(), warm_sb.ap(), mybir.ActivationFunctionType.Exp, scale=0.0
    )

    # running engine-busy estimates for greedy strip/normalize routing
    busy = {"S": 0.0, "D": 0.0}

    with tile.TileContext(nc) as tc:
        with (
            tc.tile_pool(name="cm", bufs=1) as c_pool,
            tc.tile_pool(name="qk", bufs=3) as qk_pool,
            tc.tile_pool(name="vp", bufs=3) as v_pool,
            tc.tile_pool(name="pp", bufs=8) as p_pool,
            tc.tile_pool(name="oo", bufs=8) as o_pool,
            tc.tile_pool(name="rr", bufs=8) as r_pool,
            tc.tile_pool(name="ps_s", bufs=5, space="PSUM") as ps_s,
            tc.tile_pool(name="ps_c", bufs=1, space="PSUM") as ps_c,
            tc.tile_pool(name="ps_c2", bufs=1, space="PSUM") as ps_c2,
        ):
            # constant operands for the diagonal-block causal bias matmul:
            # tri[p, m] = -8 iff m > p (else 0); ident[p, m] = (p == m).
            # matmul(sc_diag, tri, ident) accumulates -8 (= -128 in score
            # units at the 1/16 prescale) onto masked entries.
            tri_t = c_pool.tile([KB, KB], BF16, name="tri_t")
            nc.gpsimd.memset(tri_t[:], -8.0)
            nc.gpsimd.affine_select(
                out=tri_t[:],
                in_=tri_t[:],
                compare_op=mybir.AluOpType.is_gt,
                fill=0.0,
                base=0,
                pattern=[[1, KB]],
                channel_multiplier=-1,
            )
            ident_t = c_pool.tile([KB, KB], BF16, name="ident_t")
            nc.gpsimd.memset(ident_t[:], 1.0)
            nc.gpsimd.affine_select(
                out=ident_t[:],
                in_=ident_t[:],
                compare_op=mybir.AluOpType.is_equal,
                fill=0.0,
                base=0,
                pattern=[[1, KB]],
                channel_multiplier=-1,
            )
            for p in range(PPC):
                qt_t = qk_pool.tile([D, S], BF16, tag="qt")
                kt_t = qk_pool.tile([D, S], BF16, tag="kt")
                va_t = v_pool.tile([KB, NKT, KB + 1], BF16, tag="va")
                # piecewise loads (region deps) so the first score matmuls
                # only wait for the leading pieces, not the whole tensors
                for j in range(4):
                    j0, j1 = j * (S // 4), (j + 1) * (S // 4)
                    nc.sync.dma_start(out=kt_t[:, j0:j1], in_=kt_d[p][:, j0:j1])
                    nc.sync.dma_start(out=qt_t[:, j0:j1], in_=qt_d[p][:, j0:j1])
                    nc.sync.dma_start(
                        out=va_t[:, j * 4:(j + 1) * 4, :],
                        in_=va_d[p][:, j * 4:(j + 1) * 4, :],
                    )

                # last pair: big chunk first so the kernel tail is the small
                # chunk's short PV backlog
                qc_order = range(S // QC) if p < PPC - 1 else reversed(range(S // QC))
                for qc in qc_order:
                    q0 = qc * QC
                    # 8 ctx accumulators [128q, D+1], packed 3/3/2 per PSUM
                    # bank; ctx2 (stops last) double-buffered
                    ctx_tiles = [
                        ps_c.tile([128, 512], F32, tag="ctx0", name="ctx0"),
                        ps_c.tile([128, 512], F32, tag="ctx1", name="ctx1"),
                        ps_c2.tile([128, 512], F32, tag="ctx2", name="ctx2"),
                    ]

                    def ctx_ap(s):
                        t, i = divmod(s, 3)
                        return ctx_tiles[t][:, i * (KB + 1):(i + 1) * (KB + 1)]

                    nkb = (q0 + QC) // KB

                    # half-strip stream: (kb, hh) with live columns
                    # [c0, c1) of the strip; one PSUM bank per half so the
                    # score pipeline can run LOOKAHEAD halves deep,
                    # decoupling the PE round-trip latency from the exp
                    # engines.
                    halves = []
                    for kb in range(nkb):
                        lo = max(kb * KB - q0, 0)
                        for hh in range(QC // 512):
                            c0 = max(hh * 512, lo)
                            c1 = (hh + 1) * 512
                            if c0 < c1:
                                halves.append((kb, hh, c0, c1))

                    pt_tiles = {}
                    sc_tiles = {}

                    def emit_half_scores(i):
                        kb, hh, c0, c1 = halves[i]
                        k0 = kb * KB
                        off = k0 - q0
                        sc = ps_s.tile([KB, 512], F32, tag="sc", name="sc")
                        nc.tensor.matmul(
                            sc[:, c0 - hh * 512:c1 - hh * 512],
                            kt_t[:, k0:k0 + KB],
                            qt_t[:, q0 + c0:q0 + c1],
                            start=True,
                            stop=True,
                        )
                        if hh * 512 <= off < c1:
                            # causal bias on the diagonal 128x128 block
                            b0 = off - hh * 512
                            nc.tensor.matmul(
                                sc[:, b0:b0 + KB],
                                tri_t[:],
                                ident_t[:],
                                start=False,
                                stop=True,
                                skip_group_check=True,
                            )
                        sc_tiles[i] = sc

                    LOOKAHEAD = 4
                    for i in range(min(LOOKAHEAD, len(halves))):
                        emit_half_scores(i)
                    for i, (kb, hh, c0, c1) in enumerate(halves):
                        off = kb * KB - q0  # >= 0 on diagonal strips
                        sc = sc_tiles.pop(i)
                        if kb not in pt_tiles:
                            pt_tiles[kb] = p_pool.tile([KB, QC], BF16, tag="pt", name="pt")
                        pt_t = pt_tiles[kb]
                        lo_h, w = c0 - hh * 512, c1 - c0
                        has_diag = hh * 512 <= off < c1
                        # routing: the bias-masked diagonal half needs
                        # ScalarE's true exp; off-diagonal halves with >= 512
                        # keys for every row go to whichever engine is less
                        # busy (greedy min-makespan)
                        eligible = (not has_diag) and (q0 + c0) >= 512
                        cost_s = w * _NS_COL_S + _OV_S
                        cost_d = w * _NS_COL_D + _OV_D
                        if eligible and busy["D"] + cost_d <= busy["S"] + cost_s:
                            busy["D"] += cost_d
                            nc.vector._custom_dve(
                                EXP16,
                                out=pt_t[:, c0:c1],
                                in0=sc[:, lo_h:lo_h + w],
                                s0=float(B2),
                                s1=float(B1),
                                imm2=float(B0),
                            )
                        else:
                            busy["S"] += cost_s
                            nc.scalar.activation(
                                pt_t[:, c0:c1],
                                sc[:, lo_h:lo_h + w],
                                mybir.ActivationFunctionType.Exp,
                                scale=16.0,
                            )
                        if i + LOOKAHEAD < len(halves):
                            emit_half_scores(i + LOOKAHEAD)
                        # PV matmuls for the sub-q blocks of this half
                        for s in range(hh * 4, hh * 4 + 4):
                            qs0 = s * 128
                            if qs0 < c0:
                                continue  # sub-q fully masked for this k block
                            last_kb = q0 // KB + s
                            nc.tensor.matmul(
                                ctx_ap(s),
                                pt_t[:, qs0:qs0 + 128],
                                va_t[:, kb, :],
                                start=(kb == 0 and s % 3 == 0),
                                stop=(kb == last_kb),
                                skip_group_check=True,
                            )
                            # normalize + store a ctx bank as soon as its
                            # last accumulation group stopped
                            for bank, s_hi in ((0, 2), (1, 5), (2, 7)):
                                if kb != q0 // KB + s_hi or s != s_hi:
                                    continue
                                s_lo = 3 * bank
                                nsb = s_hi - s_lo + 1
                                ob = o_pool.tile([128, 3, D], F32, tag="ob")
                                rec = r_pool.tile([128, 3], F32, tag="rec")
                                # batched reciprocal of the bank's l columns
                                # (strided AP over the packed 129-col groups)
                                l_ap = ctx_tiles[bank][
                                    :, 0:nsb * (KB + 1)
                                ].rearrange("p (g c) -> p g c", c=KB + 1)[:, :, KB]
                                nc.vector.reciprocal(rec[:, 0:nsb], l_ap)
                                for s2 in range(s_lo, s_hi + 1):
                                    j = s2 - s_lo
                                    cap = ctx_ap(s2)
                                    if busy["S"] + _NORM_S <= busy["D"] + _NORM_D:
                                        busy["S"] += _NORM_S
                                        nc.scalar.activation(
                                            ob[:, j, :],
                                            cap[:, 0:D],
                                            mybir.ActivationFunctionType.Copy,
                                            scale=rec[:, j:j + 1],
                                        )
                                    else:
                                        busy["D"] += _NORM_D
                                        nc.vector.tensor_scalar_mul(
                                            ob[:, j, :],
                                            cap[:, 0:D],
                                            rec[:, j:j + 1],
                                        )
                                    # per-group store: small DMAs land on
                                    # parallel queues and start as soon as
                                    # each group is normalized
                                    nc.sync.dma_start(
                                        out=out_d[
                                            p,
                                            q0 + s2 * 128:q0 + (s2 + 1) * 128,
                                            :,
                                        ],
                                        in_=ob[:, j, :],
                                    )
    nc.compile()
    return nc


def _prep_inputs(query_layer, key_layer, value_layer):
    q = np.asarray(query_layer, dtype=np.float32).reshape(NPAIRS, S, D)
    k = np.asarray(key_layer, dtype=np.float32).reshape(NPAIRS, S, D)
    v = np.asarray(value_layer, dtype=np.float32).reshape(NPAIRS, S, D)

    qt = np.ascontiguousarray((q * np.float32(PRE)).transpose(0, 2, 1)).astype(
        ml_dtypes.bfloat16
    )
    kt = np.ascontiguousarray(k.transpose(0, 2, 1)).astype(ml_dtypes.bfloat16)
    va = np.ones((NPAIRS, KB, NKT, KB + 1), dtype=ml_dtypes.bfloat16)
    va[:, :, :, :D] = (
        v.reshape(NPAIRS, NKT, KB, D).transpose(0, 2, 1, 3).astype(ml_dtypes.bfloat16)
    )
    in_maps = [
        {
            "qt": np.ascontiguousarray(qt[c * PPC:(c + 1) * PPC]),
            "kt": np.ascontiguousarray(kt[c * PPC:(c + 1) * PPC]),
            "va": np.ascontiguousarray(va[c * PPC:(c + 1) * PPC]),
        }
        for c in range(NCORES)
    ]
    return in_maps


def _run(query_layer, key_layer, value_layer, trace=False):
    in_maps = _prep_inputs(query_layer, key_layer, value_layer)
    nc = _build_nc()
    res = run_bass_kernel_spmd(nc, in_maps, list(range(NCORES)), trace=trace)
    ctx = np.stack([res.results[c]["out"] for c in range(NCORES)])  # [8, PPC, S, D]
    out = ctx.reshape(B, H, S, D).transpose(0, 2, 1, 3).reshape(B, S, H * D)
    return np.ascontiguousarray(out, dtype=np.float32), res


def kernel(query_layer, key_layer, value_layer):
    out, _ = _run(query_layer, key_layer, value_layer, trace=False)
    return out


# revision 17
# speedup vs baseline: 1.0606x; 1.0606x over previous
"""Causal multi-head attention (B=2, H=16, S=2048, D=128, fp32) on 8 NeuronCores.

Sharding: the 32 (batch, head) pairs are split 4-per-core (tensor parallel over
heads, data parallel over batch — both collapse to the fused pair axis).

Per-core kernel (per pair), flash-attention style without max-subtraction
(scores have unit variance, so exp never overflows in fp32):

  scores_T[k, q] = K_blk^T.T @ Q^T            (bf16 matmuls into fp32 PSUM,
                                               causally trimmed free dim;
                                               Q^T prescaled by SCALE/16 so
                                               PSUM holds u = s/16)
  P_T = exp(s)                                SPLIT between ScalarE (Exp
                                              activation, scale=16) and DVE
                                              (custom EXP16_POLY_ANT op:
                                              deg-2 poly in u then 4
                                              squarings = exp(16u)·(1+2%)),
                                              strips routed by greedy balance;
                                              DVE only gets strips whose
                                              min row has >=512 keys so the
                                              softmax normalization washes
                                              out the poly ripple.
  causal mask on diagonal 128x128 blocks      (a -8 strict-upper-triangular
                                               bias is matmul-accumulated into
                                               the diagonal score block — tri
                                               stationary x identity moving —
                                               so ScalarE exp(16u-128) gives
                                               exact zeros; no mask multiply)
  ctx[q, 0:128] , l[q] = P_T_blk.T @ [V | 1]  (bf16 matmuls, PSUM-accumulated
                                               over k blocks; the ones column
                                               of V_aug yields the softmax
                                               denominator for free)
  out[q, :] = ctx[q, :] / l[q]                (DVE batched reciprocal per PSUM
                                               bank + per-group scalar multiply
                                               assigned to ScalarE or DVE by
                                               running balance)

Scheduling notes: scores for block kb+1 are emitted before PV matmuls of block
kb so the PE FIFO keeps the exp engines fed; alternating strip routing keeps
ScalarE and DVE concurrently busy. PSUM start=True clears has_written for a
whole bank, so of the 8 packed ctx accumulation groups only the first per bank
(s=0/3/6) uses start=True and the rest rely on overwrite-on-first-touch.

Q^T / K^T (bf16) and the bf16 [V | 1] augmentation are prepared host-side in
kernel() — host preprocessing is part of the sharding step.
"""

import math

import ml_dtypes
import numpy as np

import concourse.bass as bass
import concourse.mybir as mybir
from concourse import bacc, tile
from concourse.bass_utils import run_bass_kernel_spmd

# ---- custom DVE op: EXP16_POLY_ANT -----------------------------------------
# out = ((C0*u + C1)*u + C2)^16 ~= exp(16u) for u = s/16, accurate to ~2% for
# |s| <= 4.5 with graceful degradation in the tails (poly is positive
# everywhere: discriminant < 0). Registered into concourse.dve_ops at import.
from concourse.dve_spec import Spec, Src0, C0, C1, C2, lower, sq
from concourse.dve_uop import DveOpSpec
import concourse.dve_ops as dve_ops_mod
from concourse.dve_ops import DveOp

_u = Src0
_body = sq(sq(sq(sq((_u * C0 + C1) * _u + C2))))


def _exp16_ref(in0, in1, s0, s1, imm2):
    u = in0.astype(np.float32)
    p = ((u * np.float32(s0) + np.float32(s1)) * u + np.float32(imm2)).astype(
        np.float32
    )
    for _ in range(4):
        p = (p * p).astype(np.float32)
    return p


_spec = Spec(body=_body, reference=_exp16_ref)
_EXP16_NAME = "EXP16_POLY_ANT"
_EXP16_ROW = 17  # rows [1, 0x20) free; production ops use 1..16
_sha = DveOpSpec(
    name=_EXP16_NAME, opcode=_EXP16_ROW, uops=lower(_spec, ver="v3"), rd1_en=False
).sha("v3")
EXP16 = DveOp(_EXP16_NAME, _spec, subdim=False, uops_sha={"v3": _sha})
if _EXP16_NAME not in dve_ops_mod._SUB_OPCODE_FOR_NAME:
    dve_ops_mod.OPS.append(EXP16)
    dve_ops_mod.CUSTOM_DVE_SPECS[_EXP16_NAME] = _spec
    dve_ops_mod._SUB_OPCODE_FOR_NAME[_EXP16_NAME] = _EXP16_ROW

# minimax-relative fit of exp(u) on |u| <= 4.5/16 (see transcript)
B2, B1, B0 = 0.49894845, 1.0117635, 1.000231

B, H, S, D = 2, 16, 2048, 128
NCORES = 8
NPAIRS = B * H              # 32 fused (batch, head) pairs
PPC = NPAIRS // NCORES      # 4 pairs per core
KB = 128                    # k block (PE contraction / partition dim)
QC = 1024                   # q chunk (scores psum free dim)
NSUB = QC // 128            # sub-q blocks (PV stationary width) per chunk
NKT = S // KB               # 16 k blocks per sequence
SCALE = 1.0 / math.sqrt(D)  # net score scale: /(sqrt(d)*coeff) then *coeff
PRE = SCALE / 16.0          # host-side Q prescale; PSUM holds u = s/16

F32 = mybir.dt.float32
BF16 = mybir.dt.bfloat16

# modeled engine-op costs (ns) for the static balancer
_NS_COL_S = 0.833   # ScalarE activation per free column
_NS_COL_D = 1.042   # DVE per free column
_OV_S = 185.0       # ScalarE per-instruction overhead
_OV_D = 130.0       # DVE per-instruction overhead
_NORM_S = 380.0     # ScalarE Copy+scale [128,128] (measured)
_NORM_D = 322.0     # DVE tensor_scalar_mul [128,128] (measured)


def _build_nc():
    nc = bacc.Bacc("TRN2", target_bir_lowering=False, debug=False)
    qt_d = nc.dram_tensor("qt", [PPC, D, S], BF16, kind="ExternalInput")
    kt_d = nc.dram_tensor("kt", [PPC, D, S], BF16, kind="ExternalInput")
    va_d = nc.dram_tensor("va", [PPC, KB, NKT, KB + 1], BF16, kind="ExternalInput")
    out_d = nc.dram_tensor("out", [PPC, S, D], F32, kind="ExternalOutput")

    # Raw-bass warmup activation before the Tile body: places the ~1.3us ACT
    # table load in the preamble, off the first chunk's critical path.
    warm_sb = nc.alloc_sbuf_tensor("warm_sb", [128, 1], F32)
    nc.scalar.activation(
        warm_sb.ap(), warm_sb.ap(), mybir.ActivationFunctionType.Exp, scale=0.0
    )

    # running engine-busy estimates for greedy strip/normalize routing
    busy = {"S": 0.0, "D": 0.0}

    with tile.TileContext(nc) as tc:
        with (
            tc.tile_pool(name="cm", bufs=1) as c_pool,
            tc.tile_pool(name="qk", bufs=3) as qk_pool,
            tc.tile_pool(name="vp", bufs=3) as v_pool,
            tc.tile_pool(name="pp", bufs=8) as p_pool,
            tc.tile_pool(name="oo", bufs=8) as o_pool,
            tc.tile_pool(name="rr", bufs=8) as r_pool,
            tc.tile_pool(name="ps_s", bufs=5, space="PSUM") as ps_s,
            tc.tile_pool(name="ps_c", bufs=1, space="PSUM") as ps_c,
            tc.tile_pool(name="ps_c2", bufs=1, space="PSUM") as ps_c2,
        ):
            # constant operands for the diagonal-block causal bias matmul:
            # tri[p, m] = -8 iff m > p (else 0); ident[p, m] = (p == m).
            # matmul(sc_diag, tri, ident) accumulates -8 (= -128 in score
            # units at the 1/16 prescale) onto masked entries.
            tri_t = c_pool.tile([KB, KB], BF16, name="tri_t")
            nc.gpsimd.memset(tri_t[:], -8.0)
            nc.gpsimd.affine_select(
                out=tri_t[:],
                in_=tri_t[:],
                compare_op=mybir.AluOpType.is_gt,
                fill=0.0,
                base=0,
                pattern=[[1, KB]],
                channel_multiplier=-1,
            )
            ident_t = c_pool.tile([KB, KB], BF16, name="ident_t")
            nc.gpsimd.memset(ident_t[:], 1.0)
            nc.gpsimd.affine_select(
                out=ident_t[:],
                in_=ident_t[:],
                compare_op=mybir.AluOpType.is_equal,
                fill=0.0,
                base=0,
                pattern=[[1, KB]],
                channel_multiplier=-1,
            )
            for p in range(PPC):
                qt_t = qk_pool.tile([D, S], BF16, tag="qt")
                kt_t = qk_pool.tile([D, S], BF16, tag="kt")
                va_t = v_pool.tile([KB, NKT, KB + 1], BF16, tag="va")
                # asymmetric piecewise loads (region deps): a small leading
                # piece unblocks the first score matmuls early, the rest
                # follows in one transfer per tensor
                nc.sync.dma_start(out=kt_t[:, 0:256], in_=kt_d[p][:, 0:256])
                nc.sync.dma_start(out=qt_t[:, 0:1024], in_=qt_d[p][:, 0:1024])
                nc.sync.dma_start(out=va_t[:, 0:2, :], in_=va_d[p][:, 0:2, :])
                nc.sync.dma_start(out=kt_t[:, 256:], in_=kt_d[p][:, 256:])
                nc.sync.dma_start(out=qt_t[:, 1024:], in_=qt_d[p][:, 1024:])
                nc.sync.dma_start(out=va_t[:, 2:, :], in_=va_d[p][:, 2:, :])

                # last pair: big chunk first so the kernel tail is the small
                # chunk's short PV backlog
                qc_order = range(S // QC) if p < PPC - 1 else reversed(range(S // QC))
                for qc in qc_order:
                    q0 = qc * QC
                    # 8 ctx accumulators [128q, D+1], packed 3/3/2 per PSUM
                    # bank; ctx2 (stops last) double-buffered
                    ctx_tiles = [
                        ps_c.tile([128, 512], F32, tag="ctx0", name="ctx0"),
                        ps_c.tile([128, 512], F32, tag="ctx1", name="ctx1"),
                        ps_c2.tile([128, 512], F32, tag="ctx2", name="ctx2"),
                    ]

                    def ctx_ap(s):
                        t, i = divmod(s, 3)
                        return ctx_tiles[t][:, i * (KB + 1):(i + 1) * (KB + 1)]

                    nkb = (q0 + QC) // KB

                    # half-strip stream: (kb, hh) with live columns
                    # [c0, c1) of the strip; one PSUM bank per half so the
                    # score pipeline can run LOOKAHEAD halves deep,
                    # decoupling the PE round-trip latency from the exp
                    # engines.
                    halves = []
                    for kb in range(nkb):
                        lo = max(kb * KB - q0, 0)
                        for hh in range(QC // 512):
                            c0 = max(hh * 512, lo)
                            c1 = (hh + 1) * 512
                            if c0 < c1:
                                halves.append((kb, hh, c0, c1))

                    pt_tiles = {}
                    sc_tiles = {}

                    def emit_half_scores(i):
                        kb, hh, c0, c1 = halves[i]
                        k0 = kb * KB
                        off = k0 - q0
                        sc = ps_s.tile([KB, 512], F32, tag="sc", name="sc")
                        nc.tensor.matmul(
                            sc[:, c0 - hh * 512:c1 - hh * 512],
                            kt_t[:, k0:k0 + KB],
                            qt_t[:, q0 + c0:q0 + c1],
                            start=True,
                            stop=True,
                        )
                        if hh * 512 <= off < c1:
                            # causal bias on the diagonal 128x128 block
                            b0 = off - hh * 512
                            nc.tensor.matmul(
                                sc[:, b0:b0 + KB],
                                tri_t[:],
                                ident_t[:],
                                start=False,
                                stop=True,
                                skip_group_check=True,
                            )
                        sc_tiles[i] = sc

                    LOOKAHEAD = 4
                    for i in range(min(LOOKAHEAD, len(halves))):
                        emit_half_scores(i)
                    for i, (kb, hh, c0, c1) in enumerate(halves):
                        off = kb * KB - q0  # >= 0 on diagonal strips
                        sc = sc_tiles.pop(i)
                        if kb not in pt_tiles:
                            pt_tiles[kb] = p_pool.tile([KB, QC], BF16, tag="pt", name="pt")
                        pt_t = pt_tiles[kb]
                        lo_h, w = c0 - hh * 512, c1 - c0
                        has_diag = hh * 512 <= off < c1
                        # routing: the bias-masked diagonal half needs
                        # ScalarE's true exp; off-diagonal halves with >= 512
                        # keys for every row go to whichever engine is less
                        # busy (greedy min-makespan)
                        eligible = (not has_diag) and (q0 + c0) >= 512
                        cost_s = w * _NS_COL_S + _OV_S
                        cost_d = w * _NS_COL_D + _OV_D
                        if eligible and busy["D"] + cost_d <= busy["S"] + cost_s:
                            busy["D"] += cost_d
                            nc.vector._custom_dve(
                                EXP16,
                                out=pt_t[:, c0:c1],
                                in0=sc[:, lo_h:lo_h + w],
                                s0=float(B2),
                                s1=float(B1),
                                imm2=float(B0),
                            )
                        else:
                            busy["S"] += cost_s
                            nc.scalar.activation(
                                pt_t[:, c0:c1],
                                sc[:, lo_h:lo_h + w],
                                mybir.ActivationFunctionType.Exp,
                                scale=16.0,
                            )
                        if i + LOOKAHEAD < len(halves):
                            emit_half_scores(i + LOOKAHEAD)
                        # PV matmuls for the sub-q blocks of this half
                        for s in range(hh * 4, hh * 4 + 4):
                            qs0 = s * 128
                            if qs0 < c0:
                                continue  # sub-q fully masked for this k block
                            last_kb = q0 // KB + s
                            nc.tensor.matmul(
                                ctx_ap(s),
                                pt_t[:, qs0:qs0 + 128],
                                va_t[:, kb, :],
                                start=(kb == 0 and s % 3 == 0),
                                stop=(kb == last_kb),
                                skip_group_check=True,
                            )
                            # normalize + store a ctx bank as soon as its
                            # last accumulation group stopped
                            for bank, s_hi in ((0, 2), (1, 5), (2, 7)):
                                if kb != q0 // KB + s_hi or s != s_hi:
                                    continue
                                s_lo = 3 * bank
                                nsb = s_hi - s_lo + 1
                                ob = o_pool.tile([128, 3, D], F32, tag="ob")
                                rec = r_pool.tile([128, 3], F32, tag="rec")
                                # batched reciprocal of the bank's l columns
                                # (strided AP over the packed 129-col groups)
                                l_ap = ctx_tiles[bank][
                                    :, 0:nsb * (KB + 1)
                                ].rearrange("p (g c) -> p g c", c=KB + 1)[:, :, KB]
                                nc.vector.reciprocal(rec[:, 0:nsb], l_ap)
                                for s2 in range(s_lo, s_hi + 1):
                                    j = s2 - s_lo
                                    cap = ctx_ap(s2)
                                    if busy["S"] + _NORM_S <= busy["D"] + _NORM_D:
                                        busy["S"] += _NORM_S
                                        nc.scalar.activation(
                                            ob[:, j, :],
                                            cap[:, 0:D],
                                            mybir.ActivationFunctionType.Copy,
                                            scale=rec[:, j:j + 1],
                                        )
                                    else:
                                        busy["D"] += _NORM_D
                                        nc.vector.tensor_scalar_mul(
                                            ob[:, j, :],
                                            cap[:, 0:D],
                                            rec[:, j:j + 1],
                                        )
                                nc.sync.dma_start(
                                    out=out_d[
                                        p,
                                        q0 + s_lo * 128:q0 + (s_hi + 1) * 128,
                                        :,
                                    ].rearrange("(s q) d -> q s d", s=nsb),
                                    in_=ob[:, 0:nsb, :],
                                )
    nc.compile()
    return nc


def _prep_inputs(query_layer, key_layer, value_layer):
    q = np.asarray(query_layer, dtype=np.float32).reshape(NPAIRS, S, D)
    k = np.asarray(key_layer, dtype=np.float32).reshape(NPAIRS, S, D)
    v = np.asarray(value_layer, dtype=np.float32).reshape(NPAIRS, S, D)

    qt = np.ascontiguousarray((q * np.float32(PRE)).transpose(0, 2, 1)).astype(
        ml_dtypes.bfloat16
    )
    kt = np.ascontiguousarray(k.transpose(0, 2, 1)).astype(ml_dtypes.bfloat16)
    va = np.ones((NPAIRS, KB, NKT, KB + 1), dtype=ml_dtypes.bfloat16)
    va[:, :, :, :D] = (
        v.reshape(NPAIRS, NKT, KB, D).transpose(0, 2, 1, 3).astype(ml_dtypes.bfloat16)
    )
    in_maps = [
        {
            "qt": np.ascontiguousarray(qt[c * PPC:(c + 1) * PPC]),
            "kt": np.ascontiguousarray(kt[c * PPC:(c + 1) * PPC]),
            "va": np.ascontiguousarray(va[c * PPC:(c + 1) * PPC]),
        }
        for c in range(NCORES)
    ]
    return in_maps


def _run(query_layer, key_layer, value_layer, trace=False):
    in_maps = _prep_inputs(query_layer, key_layer, value_layer)
    nc = _build_nc()
    res = run_bass_kernel_spmd(nc, in_maps, list(range(NCORES)), trace=trace)
    ctx = np.stack([res.results[c]["out"] for c in range(NCORES)])  # [8, PPC, S, D]
    out = ctx.reshape(B, H, S, D).transpose(0, 2, 1, 3).reshape(B, S, H * D)
    return np.ascontiguousarray(out, dtype=np.float32), res


def kernel(query_layer, key_layer, value_layer):
    out, _ = _run(query_layer, key_layer, value_layer, trace=False)
    return out
